# revision 1
# baseline (speedup 1.0000x reference)
"""Trainium2 Bass kernel for nn_CrossAttention (cross-attention + MLP block).

Sharding: 8 cores = 2 (batch) x 4 (query-row slices of 1024). Each core runs
the full pipeline for its (b, row-slice): LN -> QKV projections -> 8-head
attention -> out-proj + residual -> LN -> MLP -> residual. kv projections are
recomputed per core (no collectives needed).

Layout strategy: activations are kept feature-major ("transposed", [feature,
token]) so every matmul chains with lhsT = weights / rhs = x^T. Attention
scores are built as S^T [m, n] so the attn@V matmul takes V as lhsT with an
appended ones-column producing the softmax denominators for free. Softmax is
computed without max-subtraction (scores are ~N(0,1): exp range is safe in
fp32, and the result is mathematically identical to the reference softmax).

dtypes: bf16 operands for all matmuls (PE runs bf16/fp32 at the same rate but
bf16 halves SBUF and weight-load cost), fp32 PSUM accumulation, fp32 LN stats
and residual stream.
"""

import os
import sys

import numpy as np

for _p in ("/opt/trn_rl_repo", os.path.expanduser("~/.axon_site/_ro/trn_rl_repo")):
    if os.path.isdir(_p) and _p not in sys.path:
        sys.path.insert(0, _p)

import ml_dtypes  # noqa: E402
from contextlib import ExitStack  # noqa: E402

import concourse.bass as bass  # noqa: E402
import concourse.tile as tile  # noqa: E402
from concourse import bacc, mybir  # noqa: E402
from concourse.bass_utils import run_bass_kernel_spmd  # noqa: E402
from concourse.masks import make_identity  # noqa: E402

B, N, M, D = 2, 4096, 4096, 512
H, DH = 8, 64
SCALE = DH ** -0.5
P = 128
NCORES = 8
RSPLIT = 4            # row-split of N per batch
NR = N // RSPLIT      # 1024 query rows per core
NT = NR // P          # 8 query row tiles
MT = M // P           # 32 kv row tiles
CC = D // P           # 4 contraction chunks of 128
G = H // 2            # 4 head groups (2 heads each; 2*64=128 partitions)
NK = NR // 512        # 2 n-chunks of 512
MC = M // 512         # 8 kv stripes of 512
VW = DH + 1           # V columns + ones column
EPS = 1e-5
F32 = mybir.dt.float32
BF16 = mybir.dt.bfloat16
AF = mybir.ActivationFunctionType
OP = mybir.AluOpType

_cache = {}


def _build_program():
    nc = bacc.Bacc("TRN2", target_bir_lowering=False, debug=False)

    q0r = nc.dram_tensor("q0r", [NR, D], F32, kind="ExternalInput")
    kv0b = nc.dram_tensor("kv0b", [M, D], F32, kind="ExternalInput")
    w_in = {
        name: nc.dram_tensor(name, [D, D], BF16, kind="ExternalInput")
        for name in ("wq", "wk", "wv", "wr", "w1", "w2")
    }
    b_in = {
        name: nc.dram_tensor(name, [D], F32, kind="ExternalInput")
        for name in ("bq", "bk", "bv", "b1", "b2")
    }
    outp = nc.dram_tensor("out", [NR, D], F32, kind="ExternalOutput")

    with tile.TileContext(nc) as tc, ExitStack() as ctx:
        consts = ctx.enter_context(tc.tile_pool(name="consts", bufs=1))
        resid = ctx.enter_context(tc.tile_pool(name="resid", bufs=1))
        proj = ctx.enter_context(tc.tile_pool(name="proj", bufs=1))

        ident = consts.tile([P, P], BF16)
        make_identity(nc, ident)
        eps_sb = consts.tile([P, 1], F32)
        nc.vector.memset(eps_sb, EPS)

        w_sb = {}
        for name, w in w_in.items():
            w_sb[name] = consts.tile([P, CC, D], BF16, tag=f"w_{name}", name=f"w_{name}")
            nc.sync.dma_start(w_sb[name], w.rearrange("(c p) i -> p c i", p=P))
        # per-inner-dim biases as [P, G] columns (inner dim on partitions)
        bcol = {}
        for name in ("bq", "bk", "b1", "b2"):
            bcol[name] = consts.tile([P, G], F32, tag=f"b_{name}", name=f"b_{name}")
            nc.sync.dma_start(bcol[name], b_in[name].rearrange("(g p) -> p g", p=P))
        # v-bias is along the free dim of natural-layout v: broadcast across partitions
        bv_bc = consts.tile([P, D], F32, tag="b_bv")
        _bv = b_in["bv"][:]
        nc.gpsimd.dma_start(
            out=bv_bc,
            in_=bass.AP(tensor=_bv.tensor, offset=_bv.offset, ap=[[0, P], *_bv.ap]),
        )

        q0_nat = resid.tile([P, NT, D], F32)
        nc.sync.dma_start(q0_nat, q0r.rearrange("(t p) d -> p t d", p=P))
        res_nat = resid.tile([P, NT, D], F32)

        kT = proj.tile([P, G, M], BF16)          # k^T, heads 2g,2g+1 on partitions
        qT = proj.tile([P, G, NR], BF16)
        v_aug = proj.tile([P, MT, H, VW], BF16)  # [V_h | 1] blocks, m on partitions
        nc.vector.memset(v_aug[:, :, :, DH:VW], 1.0)

        def layernorm_tile(pool, x_ap, tag):
            """LN over the free dim of x_ap [P, D] -> new bf16 tile."""
            stats = pool.tile([P, 6], F32, tag=f"st_{tag}")
            nc.vector.bn_stats(stats, x_ap)
            mv = pool.tile([P, 2], F32, tag=f"mv_{tag}")
            nc.vector.bn_aggr(mv, stats)
            nc.scalar.activation(mv[:, 1:2], mv[:, 1:2], AF.Sqrt, bias=eps_sb)
            nc.vector.reciprocal(mv[:, 1:2], mv[:, 1:2])
            lnx = pool.tile([P, D], BF16, tag=f"ln_{tag}")
            nc.vector.tensor_scalar(
                lnx, x_ap, mv[:, 0:1], mv[:, 1:2], op0=OP.subtract, op1=OP.mult
            )
            return lnx

        def transpose_128(psum_pool, src_ap, tag):
            """src [P, CC, P] bf16 -> psum [P, CC, P] holding the CC transposed blocks."""
            if len(src_ap.shape) == 2:
                src_ap = src_ap.rearrange("p (c q) -> p c q", q=P)
            tp = psum_pool.tile([P, CC, P], BF16, tag=tag, name=f"tp_{tag}")
            for cc in range(CC):
                nc.tensor.transpose(tp[:, cc, :], src_ap[:, cc, :], ident)
            return tp

        # ---------------- kv path: LN -> lnkv^T stripes -> kT, v ----------------
        kv_view = kv0b.rearrange("(t p) d -> p t d", p=P)
        with tc.tile_pool(name="stripes", bufs=2) as stripes, \
             tc.tile_pool(name="lnt", bufs=3) as lnt, \
             tc.tile_pool(name="pstr", bufs=2, space="PSUM") as pstr, \
             tc.tile_pool(name="pmm", bufs=2, space="PSUM") as pmm:
            for mc in range(MC):
                stripe = stripes.tile([P, CC, 512], BF16, tag="stripe")
                for j in range(4):
                    t = mc * 4 + j
                    x = lnt.tile([P, D], F32, tag="x")
                    nc.sync.dma_start(x, kv_view[:, t, :])
                    lnx = layernorm_tile(lnt, x, "kv")
                    tp = transpose_128(pstr, lnx, "tpkv")
                    nc.vector.tensor_copy(stripe[:, :, j * P:(j + 1) * P], tp)
                # kT for this 512-wide m chunk (all 4 head groups)
                for g in range(G):
                    ps = pmm.tile([P, 512], F32, tag="ps")
                    for cc in range(CC):
                        nc.tensor.matmul(
                            ps,
                            lhsT=w_sb["wk"][:, cc, g * P:(g + 1) * P],
                            rhs=stripe[:, cc, :],
                            start=(cc == 0),
                            stop=(cc == CC - 1),
                        )
                    nc.vector.tensor_scalar_add(
                        kT[:, g, mc * 512:(mc + 1) * 512], ps, bcol["bk"][:, g:g + 1]
                    )
                # v for the 4 m-tiles of this stripe
                for j in range(4):
                    t = mc * 4 + j
                    ps = pmm.tile([P, 512], F32, tag="ps")
                    for cc in range(CC):
                        nc.tensor.matmul(
                            ps,
                            lhsT=stripe[:, cc, j * P:(j + 1) * P],
                            rhs=w_sb["wv"][:, cc, :],
                            start=(cc == 0),
                            stop=(cc == CC - 1),
                        )
                    nc.vector.tensor_tensor(
                        v_aug[:, t, :, 0:DH],
                        ps.rearrange("p (h d) -> p h d", d=DH),
                        bv_bc.rearrange("p (h d) -> p h d", d=DH),
                        op=OP.add,
                    )

            # ---------------- q path: LN -> lnq^T -> qT ----------------
            lnqT = stripes.tile([P, CC, NR], BF16, tag="lnqT")
            for nt in range(NT):
                lnx = layernorm_tile(lnt, q0_nat[:, nt, :], "q")
                tp = transpose_128(pstr, lnx, "tpq")
                nc.vector.tensor_copy(lnqT[:, :, nt * P:(nt + 1) * P], tp)
            for g in range(G):
                for nk in range(NK):
                    ps = pmm.tile([P, 512], F32, tag="ps")
                    for cc in range(CC):
                        nc.tensor.matmul(
                            ps,
                            lhsT=w_sb["wq"][:, cc, g * P:(g + 1) * P],
                            rhs=lnqT[:, cc, nk * 512:(nk + 1) * 512],
                            start=(cc == 0),
                            stop=(cc == CC - 1),
                        )
                    nc.vector.tensor_scalar_add(
                        qT[:, g, nk * 512:(nk + 1) * 512], ps, bcol["bq"][:, g:g + 1]
                    )

        # ---------------- attention ----------------
        attn_pool = ctx.enter_context(tc.tile_pool(name="attn", bufs=1))
        attnT = attn_pool.tile([P, G, NR], BF16)
        with tc.tile_pool(name="ps_s", bufs=1, space="PSUM") as ps_s, \
             tc.tile_pool(name="ps_av", bufs=1, space="PSUM") as ps_av, \
             tc.tile_pool(name="ptp", bufs=2) as ptp, \
             tc.tile_pool(name="smal", bufs=2) as smal:
            # Two heads of a pair are interleaved inside the mt loop: their
            # K=64 QK matmuls sit at base partitions 0/64 -> row-tile-packed
            # on the PE, and ACT sees a continuous stream of exp tiles.
            for g in range(G):
                avs = [
                    ps_av.tile([VW, NR], F32, tag=f"av{hh}", name=f"av{hh}")
                    for hh in range(2)
                ]
                for mt in range(MT):
                    pts = []
                    for hh in range(2):
                        hp = DH * hh
                        s = ps_s.tile([P, NR], F32, tag=f"s{hh}", name=f"s{hh}")
                        for nk in range(NK):
                            nc.tensor.matmul(
                                s[:, nk * 512:(nk + 1) * 512],
                                lhsT=kT[hp:hp + DH, g, mt * P:(mt + 1) * P],
                                rhs=qT[hp:hp + DH, g, nk * 512:(nk + 1) * 512],
                                start=True,
                                stop=True,
                            )
                        pt = ptp.tile([P, NR], BF16, tag=f"pt{hh}", name=f"pt{hh}")
                        nc.scalar.activation(pt, s, AF.Exp, scale=SCALE)
                        pts.append(pt)
                    for hh in range(2):
                        for nk in range(NK):
                            nc.tensor.matmul(
                                avs[hh][:, nk * 512:(nk + 1) * 512],
                                lhsT=v_aug[:, mt, 2 * g + hh, :],
                                rhs=pts[hh][:, nk * 512:(nk + 1) * 512],
                                start=(mt == 0),
                                stop=(mt == MT - 1),
                                skip_group_check=True,
                            )
                for hh in range(2):
                    hp = DH * hh
                    recip = smal.tile([1, NR], F32, tag="recip")
                    nc.vector.reciprocal(recip, avs[hh][DH:VW, :])
                    bc = smal.tile([DH, NR], F32, tag="bc")
                    nc.gpsimd.partition_broadcast(bc, recip)
                    nc.vector.tensor_tensor(
                        attnT[hp:hp + DH, g, :], avs[hh][0:DH, :], bc, op=OP.mult
                    )

        # ---------------- out-proj + residual, MLP ----------------
        with tc.tile_pool(name="post", bufs=1) as post, \
             tc.tile_pool(name="lnt2", bufs=3) as lnt2, \
             tc.tile_pool(name="pmm2", bufs=2, space="PSUM") as pmm2, \
             tc.tile_pool(name="ptr2", bufs=2, space="PSUM") as ptr2:
            finT = post.tile([P, G, NR], BF16)
            for g in range(G):
                for nk in range(NK):
                    ps = pmm2.tile([P, 512], F32, tag="ps2")
                    for cc in range(CC):
                        nc.tensor.matmul(
                            ps,
                            lhsT=w_sb["wr"][:, cc, g * P:(g + 1) * P],
                            rhs=attnT[:, cc, nk * 512:(nk + 1) * 512],
                            start=(cc == 0),
                            stop=(cc == CC - 1),
                        )
                    nc.vector.tensor_copy(finT[:, g, nk * 512:(nk + 1) * 512], ps)
            # res = finT^T + q0
            for nt in range(NT):
                tp = transpose_128(ptr2, finT[:, :, nt * P:(nt + 1) * P], "tpf")
                nc.vector.tensor_tensor(
                    res_nat[:, nt, :],
                    tp.rearrange("p c q -> p (c q)"),
                    q0_nat[:, nt, :],
                    op=OP.add,
                )
            # MLP
            lnmT = post.tile([P, CC, NR], BF16)
            for nt in range(NT):
                lnx = layernorm_tile(lnt2, res_nat[:, nt, :], "m")
                tp = transpose_128(ptr2, lnx, "tpm")
                nc.vector.tensor_copy(lnmT[:, :, nt * P:(nt + 1) * P], tp)
            h1T = post.tile([P, G, NR], BF16)
            for g in range(G):
                for nk in range(NK):
                    ps = pmm2.tile([P, 512], F32, tag="ps2")
                    for cc in range(CC):
                        nc.tensor.matmul(
                            ps,
                            lhsT=w_sb["w1"][:, cc, g * P:(g + 1) * P],
                            rhs=lnmT[:, cc, nk * 512:(nk + 1) * 512],
                            start=(cc == 0),
                            stop=(cc == CC - 1),
                        )
                    # relu(ps + b1)
                    nc.vector.tensor_scalar(
                        h1T[:, g, nk * 512:(nk + 1) * 512],
                        ps,
                        bcol["b1"][:, g:g + 1],
                        0.0,
                        op0=OP.add,
                        op1=OP.max,
                    )
            h2T = post.tile([P, G, NR], BF16)
            for g in range(G):
                for nk in range(NK):
                    ps = pmm2.tile([P, 512], F32, tag="ps2")
                    for cc in range(CC):
                        nc.tensor.matmul(
                            ps,
                            lhsT=w_sb["w2"][:, cc, g * P:(g + 1) * P],
                            rhs=h1T[:, cc, nk * 512:(nk + 1) * 512],
                            start=(cc == 0),
                            stop=(cc == CC - 1),
                        )
                    nc.vector.tensor_scalar_add(
                        h2T[:, g, nk * 512:(nk + 1) * 512], ps, bcol["b2"][:, g:g + 1]
                    )
            # final = res + h2^T; write output
            for nt in range(NT):
                tp = transpose_128(ptr2, h2T[:, :, nt * P:(nt + 1) * P], "tph")
                nc.vector.tensor_tensor(
                    res_nat[:, nt, :],
                    tp.rearrange("p c q -> p (c q)"),
                    res_nat[:, nt, :],
                    op=OP.add,
                )
            nc.sync.dma_start(outp.rearrange("(t p) d -> p t d", p=P), res_nat)

    nc.compile()
    return nc


def _get_program():
    if "nc" not in _cache:
        _cache["nc"] = _build_program()
    return _cache["nc"]


def kernel(q0, kv0, normq_g, normq_b, normkv_g, normkv_b,
           Wq, Wk, Wv, Wr, mlp_g, mlp_b, W1, b1, W2, b2):
    q0 = np.asarray(q0, np.float32)
    kv0 = np.asarray(kv0, np.float32)
    f32 = lambda x: np.asarray(x, np.float32)
    normq_g, normq_b = f32(normq_g), f32(normq_b)
    normkv_g, normkv_b = f32(normkv_g), f32(normkv_b)
    mlp_g, mlp_b = f32(mlp_g), f32(mlp_b)
    Wq, Wk, Wv, Wr, W1, W2 = map(f32, (Wq, Wk, Wv, Wr, W1, W2))
    b1, b2 = f32(b1), f32(b2)

    # fold LN affine params into the following matmuls (exact: x_hat*g+b -> W)
    bf = lambda x: np.asarray(x, ml_dtypes.bfloat16)
    wq_e = normq_g[:, None] * Wq
    wk_e = normkv_g[:, None] * Wk
    wv_e = normkv_g[:, None] * Wv
    w1_e = mlp_g[:, None] * W1
    weights = {
        "wq": bf(wq_e), "wk": bf(wk_e), "wv": bf(wv_e),
        "wr": bf(Wr), "w1": bf(w1_e), "w2": bf(W2),
    }
    biases = {
        "bq": normq_b @ wq_e,
        "bk": normkv_b @ wk_e,
        "bv": normkv_b @ wv_e,
        "b1": (mlp_b @ w1_e) + b1,
        "b2": b2,
    }

    nc = _get_program()
    in_maps = []
    for core in range(NCORES):
        b, r = core // RSPLIT, core % RSPLIT
        m = {
            "q0r": np.ascontiguousarray(q0[b, r * NR:(r + 1) * NR]),
            "kv0b": np.ascontiguousarray(kv0[b]),
        }
        m.update(weights)
        m.update({k: np.ascontiguousarray(v) for k, v in biases.items()})
        in_maps.append(m)

    res = run_bass_kernel_spmd(nc, in_maps, list(range(NCORES)))
    out = np.empty((B, N, D), np.float32)
    for core in range(NCORES):
        b, r = core // RSPLIT, core % RSPLIT
        out[b, r * NR:(r + 1) * NR] = res.results[core]["out"]
    return out



# revision 14
# speedup vs baseline: 1.3262x; 1.3262x over previous
"""Trainium2 Bass kernel for nn_CrossAttention (cross-attention + MLP block).

Sharding: 8 cores = 2 (batch) x 4 (query-row slices of 1024). Each core runs
the full pipeline for its (b, row-slice): LN -> QKV projections -> 8-head
attention -> out-proj + residual -> LN -> MLP -> residual. kv projections are
recomputed per core (no collectives needed).

Layout strategy: activations are kept feature-major ("transposed", [feature,
token]) so every matmul chains with lhsT = weights / rhs = x^T. Attention
scores are built as S^T [m, n] so the attn@V matmul takes V as lhsT with an
appended ones-column producing the softmax denominators for free.

Softmax exp is split across three engines to unblock the scalar engine:
 - ACT: true Exp, writing fp8e5 directly (saturating).
 - DVE + GpSimd: Schraudolph fast-exp — bits = trunc(dot*A5 + B5) stored as
   uint8 and bitcast to fp8e5 (e5m2's exponent bias keeps bits >= 0 for any
   reachable score, so no clamp is needed). Softmax normalization cancels
   most of the approximation error; validated ~5e-2 at attention level,
   ~1e-3 end-to-end.

attn@V runs in fp8 DoubleRow mode: V in fp8e4 (+ ones column), exp tiles in
fp8e5, two m-tiles (K=256) per matmul at 0.5 cycles/row.

dtypes: bf16 operands elsewhere, fp32 PSUM accumulation, fp32 LN stats and
residual stream.
"""

import os
import sys

import numpy as np

for _p in ("/opt/trn_rl_repo", os.path.expanduser("~/.axon_site/_ro/trn_rl_repo")):
    if os.path.isdir(_p) and _p not in sys.path:
        sys.path.insert(0, _p)

import ml_dtypes  # noqa: E402
from contextlib import ExitStack  # noqa: E402

import concourse.bass as bass  # noqa: E402
import concourse.tile as tile  # noqa: E402
from concourse import bacc, mybir  # noqa: E402
from concourse.bass_utils import run_bass_kernel_spmd  # noqa: E402
from concourse.masks import make_identity  # noqa: E402

B, N, M, D = 2, 4096, 4096, 512
H, DH = 8, 64
SCALE = DH ** -0.5
P = 128
NCORES = 8
RSPLIT = 4            # row-split of N per batch
NR = N // RSPLIT      # 1024 query rows per core
NT = NR // P          # 8 query row tiles
MT = M // P           # 32 kv row tiles
T2 = MT // 2          # 16 kv row tile-pairs (DoubleRow planes)
CC = D // P           # 4 contraction chunks of 128
G = H // 2            # 4 head groups (2 heads each; 2*64=128 partitions)
NK = NR // 512        # 2 n-chunks of 512
MC = M // 512         # 8 kv stripes of 512
VW = DH + 1           # V columns + ones column
VWP = VW + 1          # padded so the DoubleRow plane stride is 16B-aligned
EPS = 1e-5
F32 = mybir.dt.float32
BF16 = mybir.dt.bfloat16
FP8E4 = mybir.dt.float8e4
FP8E5 = mybir.dt.float8e5
U8 = mybir.dt.uint8
AF = mybir.ActivationFunctionType
OP = mybir.AluOpType
DR = mybir.MatmulPerfMode.DoubleRow

# Schraudolph fast-exp constants for e5m2: exp(dot*SCALE) ~=
# bitcast_e5m2(trunc(dot*A5 + B5)).  A5 = SCALE * 2^2 * log2(e);
# B5 = 2^2 * 15 (exp bias) - C + 0.5 (trunc compensation).
A5 = SCALE * 4.0 / np.log(2.0)
B5 = 60.25

_cache = {}


def _build_program():
    nc = bacc.Bacc("TRN2", target_bir_lowering=False, debug=False)

    q0r = nc.dram_tensor("q0r", [NR, D], F32, kind="ExternalInput")
    kv0b = nc.dram_tensor("kv0b", [M, D], F32, kind="ExternalInput")
    w_in = {
        name: nc.dram_tensor(name, [D, D], BF16, kind="ExternalInput")
        for name in ("wq", "wk", "wv", "wr", "w1", "w2")
    }
    b_in = {
        name: nc.dram_tensor(name, [D], F32, kind="ExternalInput")
        for name in ("bq", "bk", "bv", "b1", "b2")
    }
    outp = nc.dram_tensor("out", [NR, D], F32, kind="ExternalOutput")

    with tile.TileContext(nc) as tc, ExitStack() as ctx:
        consts = ctx.enter_context(tc.tile_pool(name="consts", bufs=1))
        resid = ctx.enter_context(tc.tile_pool(name="resid", bufs=1))
        proj = ctx.enter_context(tc.tile_pool(name="proj", bufs=1))

        ident = consts.tile([P, P], BF16)
        make_identity(nc, ident)
        eps_sb = consts.tile([P, 1], F32)
        nc.vector.memset(eps_sb, EPS)

        w_sb = {}
        for name in ("wk", "wv", "wq", "wr", "w1", "w2"):  # kv-path weights first
            w = w_in[name]
            w_sb[name] = consts.tile([P, CC, D], BF16, tag=f"w_{name}", name=f"w_{name}")
            nc.sync.dma_start(w_sb[name], w.rearrange("(c p) i -> p c i", p=P))
        # per-inner-dim biases as [P, G] columns (inner dim on partitions)
        bcol = {}
        for name in ("bq", "bk", "b1", "b2"):
            bcol[name] = consts.tile([P, G], F32, tag=f"b_{name}", name=f"b_{name}")
            nc.sync.dma_start(bcol[name], b_in[name].rearrange("(g p) -> p g", p=P))
        # v-bias is along the free dim of natural-layout v: broadcast across partitions
        bv_bc = consts.tile([P, D], F32, tag="b_bv")
        _bv = b_in["bv"][:]
        nc.gpsimd.dma_start(
            out=bv_bc,
            in_=bass.AP(tensor=_bv.tensor, offset=_bv.offset, ap=[[0, P], *_bv.ap]),
        )

        q0_nat = resid.tile([P, NT, D], F32)
        res_nat = resid.tile([P, NT, D], F32)

        kT = proj.tile([P, G, M], BF16)             # k^T, heads 2g,2g+1 on partitions
        qT = proj.tile([P, G, NR], BF16)
        v_aug = proj.tile([P, T2, 2, H, VWP], FP8E4)  # [V_h | 1] DoubleRow planes
        nc.vector.memset(v_aug[:, :, :, :, DH:VW], 1.0)

        def layernorm_tile(pool, x_ap, tag, stats_eng=None):
            """LN over the free dim of x_ap [P, D] -> new bf16 tile.
            Stats on DVE (or GpSimd), the big normalize on ACT
            (x*rstd + (-mu*rstd))."""
            stats = pool.tile([P, 6], F32, tag=f"st_{tag}")
            (stats_eng or nc.vector).bn_stats(stats, x_ap)
            mv = pool.tile([P, 2], F32, tag=f"mv_{tag}")
            nc.vector.bn_aggr(mv, stats)
            nc.scalar.activation(mv[:, 1:2], mv[:, 1:2], AF.Sqrt, bias=eps_sb)
            nc.vector.reciprocal(mv[:, 1:2], mv[:, 1:2])
            nm = pool.tile([P, 1], F32, tag=f"nm_{tag}")
            nc.vector.tensor_scalar(
                nm, mv[:, 0:1], mv[:, 1:2], -1.0, op0=OP.mult, op1=OP.mult
            )
            lnx = pool.tile([P, D], BF16, tag=f"ln_{tag}")
            nc.scalar.activation(lnx, x_ap, AF.Identity, bias=nm, scale=mv[:, 1:2])
            return lnx

        def transpose_128(psum_pool, src_ap, tag):
            """src [P, CC, P] bf16 -> psum [P, CC, P] holding the CC transposed blocks."""
            if len(src_ap.shape) == 2:
                src_ap = src_ap.rearrange("p (c q) -> p c q", q=P)
            tp = psum_pool.tile([P, CC, P], BF16, tag=tag, name=f"tp_{tag}")
            for cc in range(CC):
                nc.tensor.transpose(tp[:, cc, :], src_ap[:, cc, :], ident)
            return tp

        # ---------------- kv path: LN -> lnkv^T stripes -> kT, v ----------------
        kv_view = kv0b.rearrange("(t p) d -> p t d", p=P)
        with tc.tile_pool(name="stripes", bufs=2) as stripes, \
             tc.tile_pool(name="lnt", bufs=3) as lnt, \
             tc.tile_pool(name="pstr", bufs=2, space="PSUM") as pstr, \
             tc.tile_pool(name="pmm", bufs=2, space="PSUM") as pmm:
            for mc in range(MC):
                stripe = stripes.tile([P, CC, 512], BF16, tag="stripe")
                for j in range(4):
                    t = mc * 4 + j
                    x = lnt.tile([P, D], F32, tag="x")
                    nc.sync.dma_start(x, kv_view[:, t, :])
                    lnx = layernorm_tile(lnt, x, "kv")
                    tp = transpose_128(pstr, lnx, "tpkv")
                    nc.vector.tensor_copy(stripe[:, :, j * P:(j + 1) * P], tp)
                # kT for this 512-wide m chunk (all 4 head groups)
                for g in range(G):
                    ps = pmm.tile([P, 512], F32, tag="ps")
                    for cc in range(CC):
                        nc.tensor.matmul(
                            ps,
                            lhsT=w_sb["wk"][:, cc, g * P:(g + 1) * P],
                            rhs=stripe[:, cc, :],
                            start=(cc == 0),
                            stop=(cc == CC - 1),
                        )
                    nc.scalar.activation(
                        kT[:, g, mc * 512:(mc + 1) * 512], ps, AF.Identity,
                        bias=bcol["bk"][:, g:g + 1],
                    )
                # v for the 4 m-tiles of this stripe
                for j in range(4):
                    t = mc * 4 + j
                    ps = pmm.tile([P, 512], F32, tag="ps")
                    for cc in range(CC):
                        nc.tensor.matmul(
                            ps,
                            lhsT=stripe[:, cc, j * P:(j + 1) * P],
                            rhs=w_sb["wv"][:, cc, :],
                            start=(cc == 0),
                            stop=(cc == CC - 1),
                        )
                    nc.vector.tensor_tensor(
                        v_aug[:, t // 2, t % 2, :, 0:DH],
                        ps.rearrange("p (h d) -> p h d", d=DH),
                        bv_bc.rearrange("p (h d) -> p h d", d=DH),
                        op=OP.add,
                    )

            # ---------------- q path: LN -> lnq^T -> qT ----------------
            nc.sync.dma_start(q0_nat, q0r.rearrange("(t p) d -> p t d", p=P))
            lnqT = stripes.tile([P, CC, NR], BF16, tag="lnqT")
            for nt in range(NT):
                lnx = layernorm_tile(lnt, q0_nat[:, nt, :], "q")
                tp = transpose_128(pstr, lnx, "tpq")
                nc.vector.tensor_copy(lnqT[:, :, nt * P:(nt + 1) * P], tp)
            for g in range(G):
                for nk in range(NK):
                    ps = pmm.tile([P, 512], F32, tag="ps")
                    for cc in range(CC):
                        nc.tensor.matmul(
                            ps,
                            lhsT=w_sb["wq"][:, cc, g * P:(g + 1) * P],
                            rhs=lnqT[:, cc, nk * 512:(nk + 1) * 512],
                            start=(cc == 0),
                            stop=(cc == CC - 1),
                        )
                    nc.scalar.activation(
                        qT[:, g, nk * 512:(nk + 1) * 512], ps, AF.Identity,
                        bias=bcol["bq"][:, g:g + 1],
                    )

        # ---------------- attention ----------------
        # Per (g, t2): 4 units of (2 QK matmuls -> one [128, NR] exp), split
        # ACT (true Exp -> fp8e5) / DVE (Schraudolph -> uint8 bitcast fp8e5);
        # then 4 DoubleRow attn@V matmuls (K=256 over the two m-planes).
        # GpSimd cannot read PSUM, so it sits out of the exp rotation.
        # ACT:DVE unit ratio 2.25:1.75 balances (512+172)/1.2 vs (512+120)/.96
        # plus DVE's per-g reciprocal+normalize work.
        attn_pool = ctx.enter_context(tc.tile_pool(name="attn", bufs=1))
        attnT = attn_pool.tile([P, G, NR], BF16)
        with tc.tile_pool(name="ps_s", bufs=1, space="PSUM") as ps_s, \
             tc.tile_pool(name="ps_av", bufs=1, space="PSUM") as ps_av, \
             tc.tile_pool(name="ptp", bufs=2) as ptp, \
             tc.tile_pool(name="smal", bufs=2) as smal:
            DVESET = frozenset({1, 3, 5, 7, 9, 11, 13})  # 7 of 16 units -> DVE
            for g in range(G):
                for nk in range(NK):
                    nks = slice(nk * 512, (nk + 1) * 512)
                    avs = [
                        ps_av.tile([VW, 512], F32, tag=f"av{hh}", name=f"av{hh}")
                        for hh in range(2)
                    ]
                    for t2 in range(T2):
                        pts = [
                            ptp.tile([P, 2, 512], FP8E5, tag=f"pt{hh}",
                                     name=f"pt{hh}")
                            for hh in range(2)
                        ]
                        for hh in range(2):
                            hp = DH * hh
                            u = t2 * 2 + hh
                            s = ps_s.tile([P, 2, 512], F32, tag=f"s{u % 3}")
                            for pl in range(2):
                                mt = 2 * t2 + pl
                                nc.tensor.matmul(
                                    s[:, pl, :],
                                    lhsT=kT[hp:hp + DH, g, mt * P:(mt + 1) * P],
                                    rhs=qT[hp:hp + DH, g, nks],
                                    start=True,
                                    stop=True,
                                )
                            # one fused exp over both m-planes [128, 1024]
                            dst = pts[hh][:, :, :]
                            if u % 16 in DVESET:
                                nc.vector.tensor_scalar(
                                    dst.bitcast(U8), s, A5, B5,
                                    op0=OP.mult, op1=OP.add,
                                )
                            else:
                                nc.scalar.activation(dst, s, AF.Exp, scale=SCALE)
                        for hh in range(2):
                            nc.tensor.matmul(
                                avs[hh],
                                lhsT=v_aug[:, t2, :, 2 * g + hh, 0:VW],
                                rhs=pts[hh],
                                start=(t2 == 0),
                                stop=(t2 == T2 - 1),
                                perf_mode=DR,
                                skip_group_check=True,
                            )
                    for hh in range(2):
                        hp = DH * hh
                        recip = smal.tile([1, 512], F32, tag="recip")
                        nc.vector.reciprocal(recip, avs[hh][DH:VW, :])
                        bc = smal.tile([DH, 512], F32, tag="bc")
                        nc.gpsimd.partition_broadcast(bc, recip)
                        nc.vector.tensor_tensor(
                            attnT[hp:hp + DH, g, nks], avs[hh][0:DH, :],
                            bc, op=OP.mult
                        )

        # ---------------- out-proj + residual, MLP ----------------
        with tc.tile_pool(name="post", bufs=1) as post, \
             tc.tile_pool(name="lnt2", bufs=3) as lnt2, \
             tc.tile_pool(name="pmm2", bufs=2, space="PSUM") as pmm2, \
             tc.tile_pool(name="ptr2", bufs=2, space="PSUM") as ptr2:
            finT = post.tile([P, G, NR], BF16)
            for g in range(G):
                for nk in range(NK):
                    ps = pmm2.tile([P, 512], F32, tag="ps2")
                    for cc in range(CC):
                        nc.tensor.matmul(
                            ps,
                            lhsT=w_sb["wr"][:, cc, g * P:(g + 1) * P],
                            rhs=attnT[:, cc, nk * 512:(nk + 1) * 512],
                            start=(cc == 0),
                            stop=(cc == CC - 1),
                        )
                    nc.scalar.copy(finT[:, g, nk * 512:(nk + 1) * 512], ps)
            # res = finT^T + q0
            for nt in range(NT):
                tp = transpose_128(ptr2, finT[:, :, nt * P:(nt + 1) * P], "tpf")
                nc.vector.tensor_tensor(
                    res_nat[:, nt, :],
                    tp.rearrange("p c q -> p (c q)"),
                    q0_nat[:, nt, :],
                    op=OP.add,
                )
            # MLP
            lnmT = post.tile([P, CC, NR], BF16)
            for nt in range(NT):
                lnx = layernorm_tile(lnt2, res_nat[:, nt, :], "m")
                tp = transpose_128(ptr2, lnx, "tpm")
                nc.vector.tensor_copy(lnmT[:, :, nt * P:(nt + 1) * P], tp)
            h1T = post.tile([P, G, NR], BF16)
            for g in range(G):
                for nk in range(NK):
                    ps = pmm2.tile([P, 512], F32, tag="ps2")
                    for cc in range(CC):
                        nc.tensor.matmul(
                            ps,
                            lhsT=w_sb["w1"][:, cc, g * P:(g + 1) * P],
                            rhs=lnmT[:, cc, nk * 512:(nk + 1) * 512],
                            start=(cc == 0),
                            stop=(cc == CC - 1),
                        )
                    nc.scalar.activation(
                        h1T[:, g, nk * 512:(nk + 1) * 512], ps, AF.Relu,
                        bias=bcol["b1"][:, g:g + 1],
                    )
            h2T = post.tile([P, G, NR], BF16)
            for g in range(G):
                for nk in range(NK):
                    ps = pmm2.tile([P, 512], F32, tag="ps2")
                    for cc in range(CC):
                        nc.tensor.matmul(
                            ps,
                            lhsT=w_sb["w2"][:, cc, g * P:(g + 1) * P],
                            rhs=h1T[:, cc, nk * 512:(nk + 1) * 512],
                            start=(cc == 0),
                            stop=(cc == CC - 1),
                        )
                    nc.vector.tensor_scalar_add(
                        h2T[:, g, nk * 512:(nk + 1) * 512], ps, bcol["b2"][:, g:g + 1]
                    )
            # final = res + h2^T; stream each tile out as it completes
            outp_v = outp.rearrange("(t p) d -> p t d", p=P)
            for nt in range(NT):
                tp = transpose_128(ptr2, h2T[:, :, nt * P:(nt + 1) * P], "tph")
                nc.vector.tensor_tensor(
                    res_nat[:, nt, :],
                    tp.rearrange("p c q -> p (c q)"),
                    res_nat[:, nt, :],
                    op=OP.add,
                )
                nc.sync.dma_start(outp_v[:, nt, :], res_nat[:, nt, :])

    nc.compile()
    return nc


def _get_program():
    if "nc" not in _cache:
        _cache["nc"] = _build_program()
    return _cache["nc"]


def kernel(q0, kv0, normq_g, normq_b, normkv_g, normkv_b,
           Wq, Wk, Wv, Wr, mlp_g, mlp_b, W1, b1, W2, b2):
    q0 = np.asarray(q0, np.float32)
    kv0 = np.asarray(kv0, np.float32)
    f32 = lambda x: np.asarray(x, np.float32)
    normq_g, normq_b = f32(normq_g), f32(normq_b)
    normkv_g, normkv_b = f32(normkv_g), f32(normkv_b)
    mlp_g, mlp_b = f32(mlp_g), f32(mlp_b)
    Wq, Wk, Wv, Wr, W1, W2 = map(f32, (Wq, Wk, Wv, Wr, W1, W2))
    b1, b2 = f32(b1), f32(b2)

    # fold LN affine params into the following matmuls (exact: x_hat*g+b -> W)
    bf = lambda x: np.asarray(x, ml_dtypes.bfloat16)
    wq_e = normq_g[:, None] * Wq
    wk_e = normkv_g[:, None] * Wk
    wv_e = normkv_g[:, None] * Wv
    w1_e = mlp_g[:, None] * W1
    weights = {
        "wq": bf(wq_e), "wk": bf(wk_e), "wv": bf(wv_e),
        "wr": bf(Wr), "w1": bf(w1_e), "w2": bf(W2),
    }
    biases = {
        "bq": normq_b @ wq_e,
        "bk": normkv_b @ wk_e,
        "bv": normkv_b @ wv_e,
        "b1": (mlp_b @ w1_e) + b1,
        "b2": b2,
    }

    nc = _get_program()
    in_maps = []
    for core in range(NCORES):
        b, r = core // RSPLIT, core % RSPLIT
        m = {
            "q0r": np.ascontiguousarray(q0[b, r * NR:(r + 1) * NR]),
            "kv0b": np.ascontiguousarray(kv0[b]),
        }
        m.update(weights)
        m.update({k: np.ascontiguousarray(v) for k, v in biases.items()})
        in_maps.append(m)

    res = run_bass_kernel_spmd(nc, in_maps, list(range(NCORES)))
    out = np.empty((B, N, D), np.float32)
    for core in range(NCORES):
        b, r = core // RSPLIT, core % RSPLIT
        out[b, r * NR:(r + 1) * NR] = res.results[core]["out"]
    return out


# revision 29
# speedup vs baseline: 1.4209x; 1.0714x over previous
"""Trainium2 Bass kernel for nn_CrossAttention (cross-attention + MLP block).

Sharding: 8 cores = 2 (batch) x 4 (query-row slices of 1024). Each core runs
the full pipeline for its (b, row-slice): LN -> QKV projections -> 8-head
attention -> out-proj + residual -> LN -> MLP -> residual. kv projections are
recomputed per core (no collectives needed).

Layout strategy: activations are kept feature-major ("transposed", [feature,
token]) so every matmul chains with lhsT = weights / rhs = x^T. Attention
scores are built as S^T [m, n] so the attn@V matmul takes V as lhsT with an
appended ones-column producing the softmax denominators for free.

Softmax exp is split across three engines to unblock the scalar engine:
 - ACT: true Exp, writing fp8e5 directly (saturating).
 - DVE + GpSimd: Schraudolph fast-exp — bits = trunc(dot*A5 + B5) stored as
   uint8 and bitcast to fp8e5 (e5m2's exponent bias keeps bits >= 0 for any
   reachable score, so no clamp is needed). Softmax normalization cancels
   most of the approximation error; validated ~5e-2 at attention level,
   ~1e-3 end-to-end.

attn@V runs in fp8 DoubleRow mode: V in fp8e4 (+ ones column), exp tiles in
fp8e5, two m-tiles (K=256) per matmul at 0.5 cycles/row.

dtypes: bf16 operands elsewhere, fp32 PSUM accumulation, fp32 LN stats and
residual stream.
"""

import os
import sys

import numpy as np

for _p in ("/opt/trn_rl_repo", os.path.expanduser("~/.axon_site/_ro/trn_rl_repo")):
    if os.path.isdir(_p) and _p not in sys.path:
        sys.path.insert(0, _p)

import ml_dtypes  # noqa: E402
from contextlib import ExitStack  # noqa: E402

import concourse.bass as bass  # noqa: E402
import concourse.tile as tile  # noqa: E402
from concourse import bacc, mybir  # noqa: E402
from concourse.bass_utils import run_bass_kernel_spmd  # noqa: E402
from concourse.masks import make_identity  # noqa: E402

B, N, M, D = 2, 4096, 4096, 512
H, DH = 8, 64
SCALE = DH ** -0.5
P = 128
NCORES = 8
RSPLIT = 4            # row-split of N per batch
NR = N // RSPLIT      # 1024 query rows per core
NT = NR // P          # 8 query row tiles
MT = M // P           # 32 kv row tiles
T2 = MT // 2          # 16 kv row tile-pairs (DoubleRow planes)
CC = D // P           # 4 contraction chunks of 128
G = H // 2            # 4 head groups (2 heads each; 2*64=128 partitions)
NK = NR // 512        # 2 n-chunks of 512
MC = M // 512         # 8 kv stripes of 512
VW = DH + 1           # V columns + ones column
VWP = VW + 1          # padded so the DoubleRow plane stride is 16B-aligned
EPS = 1e-5
F32 = mybir.dt.float32
BF16 = mybir.dt.bfloat16
FP8E4 = mybir.dt.float8e4
FP8E5 = mybir.dt.float8e5
U8 = mybir.dt.uint8
AF = mybir.ActivationFunctionType
OP = mybir.AluOpType
DR = mybir.MatmulPerfMode.DoubleRow

# Schraudolph fast-exp constants for e5m2: exp(dot*SCALE) ~=
# bitcast_e5m2(trunc(dot*A5 + B5)).  A5 = SCALE * 2^2 * log2(e);
# B5 = 2^2 * 15 (exp bias) - C + 0.5 (trunc compensation).
A5 = SCALE * 4.0 / np.log(2.0)
B5 = 60.25

_cache = {}


def _build_program():
    nc = bacc.Bacc("TRN2", target_bir_lowering=False, debug=False)

    q0r = nc.dram_tensor("q0r", [NR, D], F32, kind="ExternalInput")
    kv0b = nc.dram_tensor("kv0b", [M, D], F32, kind="ExternalInput")
    w_in = {
        name: nc.dram_tensor(name, [D, D], BF16, kind="ExternalInput")
        for name in ("wq", "wk", "wv", "wr", "w1", "w2")
    }
    b_in = {
        name: nc.dram_tensor(name, [D], F32, kind="ExternalInput")
        for name in ("bq", "bk", "bv", "b1", "b2")
    }
    outp = nc.dram_tensor("out", [NR, D], F32, kind="ExternalOutput")

    with tile.TileContext(nc) as tc, ExitStack() as ctx:
        consts = ctx.enter_context(tc.tile_pool(name="consts", bufs=1))
        resid = ctx.enter_context(tc.tile_pool(name="resid", bufs=1))
        proj = ctx.enter_context(tc.tile_pool(name="proj", bufs=1))

        ident = consts.tile([P, P], BF16)
        make_identity(nc, ident)
        eps_sb = consts.tile([P, 1], F32)
        nc.vector.memset(eps_sb, EPS)

        w_sb = {}
        for name in ("wk", "wv", "wq", "wr", "w1", "w2"):
            w_sb[name] = consts.tile([P, CC, D], BF16, tag=f"w_{name}", name=f"w_{name}")

        def load_w(name):
            nc.sync.dma_start(
                w_sb[name], w_in[name].rearrange("(c p) i -> p c i", p=P)
            )

        # only the kv-path weights up front; the rest stream in later so they
        # don't head-of-line-block the kv activations at startup
        load_w("wk")
        load_w("wv")
        # per-inner-dim biases as [P, G] columns (inner dim on partitions)
        bcol = {}
        for name in ("bq", "bk", "b1", "b2"):
            bcol[name] = consts.tile([P, G], F32, tag=f"b_{name}", name=f"b_{name}")
            nc.sync.dma_start(bcol[name], b_in[name].rearrange("(g p) -> p g", p=P))
        # v-bias is along the free dim of natural-layout v: broadcast across partitions
        bv_bc = consts.tile([P, D], F32, tag="b_bv")
        _bv = b_in["bv"][:]
        nc.gpsimd.dma_start(
            out=bv_bc,
            in_=bass.AP(tensor=_bv.tensor, offset=_bv.offset, ap=[[0, P], *_bv.ap]),
        )

        q0_nat = resid.tile([P, NT, D], F32)
        res_nat = resid.tile([P, NT, D], F32)

        kT = proj.tile([P, G, M], BF16)             # k^T, heads 2g,2g+1 on partitions
        qT = proj.tile([P, G, NR], BF16)
        v_aug = proj.tile([P, T2, 2, H, VWP], FP8E4)  # [V_h | 1] DoubleRow planes
        nc.vector.memset(v_aug[:, :, :, :, DH:VW], 1.0)

        def layernorm_tile(pool, x_ap, tag, stats_eng=None):
            """LN over the free dim of x_ap [P, D] -> new bf16 tile.
            Stats on DVE (or GpSimd), the big normalize on ACT
            (x*rstd + (-mu*rstd))."""
            stats = pool.tile([P, 6], F32, tag=f"st_{tag}")
            (stats_eng or nc.vector).bn_stats(stats, x_ap)
            mv = pool.tile([P, 2], F32, tag=f"mv_{tag}")
            nc.vector.bn_aggr(mv, stats)
            nc.scalar.activation(mv[:, 1:2], mv[:, 1:2], AF.Sqrt, bias=eps_sb)
            nc.vector.reciprocal(mv[:, 1:2], mv[:, 1:2])
            nm = pool.tile([P, 1], F32, tag=f"nm_{tag}")
            nc.vector.tensor_scalar(
                nm, mv[:, 0:1], mv[:, 1:2], -1.0, op0=OP.mult, op1=OP.mult
            )
            lnx = pool.tile([P, D], BF16, tag=f"ln_{tag}")
            nc.scalar.activation(lnx, x_ap, AF.Identity, bias=nm, scale=mv[:, 1:2])
            return lnx

        def transpose_128(psum_pool, src_ap, tag):
            """src [P, CC, P] bf16 -> psum [P, CC, P] holding the CC transposed blocks."""
            if len(src_ap.shape) == 2:
                src_ap = src_ap.rearrange("p (c q) -> p c q", q=P)
            tp = psum_pool.tile([P, CC, P], BF16, tag=tag, name=f"tp_{tag}")
            for cc in range(CC):
                nc.tensor.transpose(tp[:, cc, :], src_ap[:, cc, :], ident)
            return tp

        # ---------------- kv path: LN -> lnkv^T stripes -> kT, v ----------------
        kv_view = kv0b.rearrange("(t p) d -> p t d", p=P)
        with tc.tile_pool(name="stripes", bufs=3) as stripes, \
             tc.tile_pool(name="lnt", bufs=8) as lnt, \
             tc.tile_pool(name="pstr", bufs=2, space="PSUM") as pstr, \
             tc.tile_pool(name="pmm", bufs=2, space="PSUM") as pmm:
            for mc in range(MC):
                stripe = stripes.tile([P, CC, 512], BF16, tag="stripe")
                for j in range(4):
                    t = mc * 4 + j
                    x = lnt.tile([P, D], F32, tag="x")
                    nc.sync.dma_start(x, kv_view[:, t, :])
                    lnx = layernorm_tile(lnt, x, "kv")
                    tp = transpose_128(pstr, lnx, "tpkv")
                    nc.vector.tensor_copy(stripe[:, :, j * P:(j + 1) * P], tp)
                # kT for this 512-wide m chunk (all 4 head groups)
                for g in range(G):
                    ps = pmm.tile([P, 512], F32, tag="ps")
                    for cc in range(CC):
                        nc.tensor.matmul(
                            ps,
                            lhsT=w_sb["wk"][:, cc, g * P:(g + 1) * P],
                            rhs=stripe[:, cc, :],
                            start=(cc == 0),
                            stop=(cc == CC - 1),
                        )
                    nc.scalar.activation(
                        kT[:, g, mc * 512:(mc + 1) * 512], ps, AF.Identity,
                        bias=bcol["bk"][:, g:g + 1],
                    )
                # v for the 4 m-tiles of this stripe
                for j in range(4):
                    t = mc * 4 + j
                    ps = pmm.tile([P, 512], F32, tag="ps")
                    for cc in range(CC):
                        nc.tensor.matmul(
                            ps,
                            lhsT=stripe[:, cc, j * P:(j + 1) * P],
                            rhs=w_sb["wv"][:, cc, :],
                            start=(cc == 0),
                            stop=(cc == CC - 1),
                        )
                    nc.vector.tensor_tensor(
                        v_aug[:, t // 2, t % 2, :, 0:DH],
                        ps.rearrange("p (h d) -> p h d", d=DH),
                        bv_bc.rearrange("p (h d) -> p h d", d=DH),
                        op=OP.add,
                    )

            # ---------------- q path: LN -> lnq^T -> qT ----------------
            load_w("wq")
            load_w("wr")
            load_w("w1")
            load_w("w2")
            nc.sync.dma_start(q0_nat, q0r.rearrange("(t p) d -> p t d", p=P))
            lnqT = stripes.tile([P, CC, NR], BF16, tag="lnqT")
            for nt in range(NT):
                lnx = layernorm_tile(lnt, q0_nat[:, nt, :], "q")
                tp = transpose_128(pstr, lnx, "tpq")
                nc.vector.tensor_copy(lnqT[:, :, nt * P:(nt + 1) * P], tp)
            for g in range(G):
                for nk in range(NK):
                    ps = pmm.tile([P, 512], F32, tag="ps")
                    for cc in range(CC):
                        nc.tensor.matmul(
                            ps,
                            lhsT=w_sb["wq"][:, cc, g * P:(g + 1) * P],
                            rhs=lnqT[:, cc, nk * 512:(nk + 1) * 512],
                            start=(cc == 0),
                            stop=(cc == CC - 1),
                        )
                    nc.scalar.activation(
                        qT[:, g, nk * 512:(nk + 1) * 512], ps, AF.Identity,
                        bias=bcol["bq"][:, g:g + 1],
                    )

        # ---------------- attention ----------------
        # Per (g, t2): 4 units of (2 QK matmuls -> one [128, NR] exp), split
        # ACT (true Exp -> fp8e5) / DVE (Schraudolph -> uint8 bitcast fp8e5);
        # then 4 DoubleRow attn@V matmuls (K=256 over the two m-planes).
        # GpSimd cannot read PSUM, so it sits out of the exp rotation.
        # ACT:DVE unit ratio 2.25:1.75 balances (512+172)/1.2 vs (512+120)/.96
        # plus DVE's per-g reciprocal+normalize work.
        attn_pool = ctx.enter_context(tc.tile_pool(name="attn", bufs=1))
        attnT = attn_pool.tile([P, G, NR], BF16)
        with tc.tile_pool(name="ps_s", bufs=1, space="PSUM") as ps_s, \
             tc.tile_pool(name="ps_av", bufs=1, space="PSUM") as ps_av, \
             tc.tile_pool(name="ptp", bufs=3) as ptp, \
             tc.tile_pool(name="smal", bufs=3) as smal:
            DVESET = frozenset({1, 3, 5, 7, 9, 11, 13})  # 7 of 16 units -> DVE
            for g in range(G):
                for nk in range(NK):
                    nks = slice(nk * 512, (nk + 1) * 512)
                    avs = [
                        ps_av.tile([VW, 512], F32, tag=f"av{hh}", name=f"av{hh}")
                        for hh in range(2)
                    ]
                    for t2 in range(T2):
                        pts = [
                            ptp.tile([P, 2, 512], FP8E5, tag=f"pt{hh}",
                                     name=f"pt{hh}")
                            for hh in range(2)
                        ]
                        for hh in range(2):
                            hp = DH * hh
                            u = t2 * 2 + hh
                            s = ps_s.tile([P, 2, 512], F32, tag=f"s{u % 3}")
                            for pl in range(2):
                                mt = 2 * t2 + pl
                                nc.tensor.matmul(
                                    s[:, pl, :],
                                    lhsT=kT[hp:hp + DH, g, mt * P:(mt + 1) * P],
                                    rhs=qT[hp:hp + DH, g, nks],
                                    start=True,
                                    stop=True,
                                )
                            # one fused exp over both m-planes [128, 1024]
                            dst = pts[hh][:, :, :]
                            if u % 16 in DVESET:
                                nc.vector.tensor_scalar(
                                    dst.bitcast(U8), s, A5, B5,
                                    op0=OP.mult, op1=OP.add,
                                )
                            else:
                                nc.scalar.activation(dst, s, AF.Exp, scale=SCALE)
                        for hh in range(2):
                            nc.tensor.matmul(
                                avs[hh],
                                lhsT=v_aug[:, t2, :, 2 * g + hh, 0:VW],
                                rhs=pts[hh],
                                start=(t2 == 0),
                                stop=(t2 == T2 - 1),
                                perf_mode=DR,
                                skip_group_check=True,
                            )
                    for hh in range(2):
                        hp = DH * hh
                        recip = smal.tile([1, 512], F32, tag="recip")
                        nc.vector.reciprocal(recip, avs[hh][DH:VW, :])
                        bc = smal.tile([DH, 512], F32, tag="bc")
                        nc.gpsimd.partition_broadcast(bc, recip)
                        nc.vector.tensor_tensor(
                            attnT[hp:hp + DH, g, nks], avs[hh][0:DH, :],
                            bc, op=OP.mult
                        )

        # ---------------- out-proj + residual, MLP ----------------
        with tc.tile_pool(name="post", bufs=1) as post, \
             tc.tile_pool(name="lnt2", bufs=8) as lnt2, \
             tc.tile_pool(name="pmm2", bufs=2, space="PSUM") as pmm2, \
             tc.tile_pool(name="ptr2", bufs=2, space="PSUM") as ptr2:
            finT = post.tile([P, G, NR], BF16)
            for g in range(G):
                for nk in range(NK):
                    ps = pmm2.tile([P, 512], F32, tag="ps2")
                    for cc in range(CC):
                        nc.tensor.matmul(
                            ps,
                            lhsT=w_sb["wr"][:, cc, g * P:(g + 1) * P],
                            rhs=attnT[:, cc, nk * 512:(nk + 1) * 512],
                            start=(cc == 0),
                            stop=(cc == CC - 1),
                        )
                    nc.scalar.copy(finT[:, g, nk * 512:(nk + 1) * 512], ps)
            # res = finT^T + q0
            for nt in range(NT):
                tp = transpose_128(ptr2, finT[:, :, nt * P:(nt + 1) * P], "tpf")
                nc.vector.tensor_tensor(
                    res_nat[:, nt, :],
                    tp.rearrange("p c q -> p (c q)"),
                    q0_nat[:, nt, :],
                    op=OP.add,
                )
            # MLP
            lnmT = post.tile([P, CC, NR], BF16)
            for nt in range(NT):
                lnx = layernorm_tile(lnt2, res_nat[:, nt, :], "m")
                tp = transpose_128(ptr2, lnx, "tpm")
                nc.vector.tensor_copy(lnmT[:, :, nt * P:(nt + 1) * P], tp)
            h1T = post.tile([P, G, NR], BF16)
            for g in range(G):
                for nk in range(NK):
                    ps = pmm2.tile([P, 512], F32, tag="ps2")
                    for cc in range(CC):
                        nc.tensor.matmul(
                            ps,
                            lhsT=w_sb["w1"][:, cc, g * P:(g + 1) * P],
                            rhs=lnmT[:, cc, nk * 512:(nk + 1) * 512],
                            start=(cc == 0),
                            stop=(cc == CC - 1),
                        )
                    nc.scalar.activation(
                        h1T[:, g, nk * 512:(nk + 1) * 512], ps, AF.Relu,
                        bias=bcol["b1"][:, g:g + 1],
                    )
            h2T = post.tile([P, G, NR], BF16)
            for g in range(G):
                for nk in range(NK):
                    ps = pmm2.tile([P, 512], F32, tag="ps2")
                    for cc in range(CC):
                        nc.tensor.matmul(
                            ps,
                            lhsT=w_sb["w2"][:, cc, g * P:(g + 1) * P],
                            rhs=h1T[:, cc, nk * 512:(nk + 1) * 512],
                            start=(cc == 0),
                            stop=(cc == CC - 1),
                        )
                    nc.vector.tensor_scalar_add(
                        h2T[:, g, nk * 512:(nk + 1) * 512], ps, bcol["b2"][:, g:g + 1]
                    )
            # final = res + h2^T; stream each tile out as it completes
            outp_v = outp.rearrange("(t p) d -> p t d", p=P)
            for nt in range(NT):
                tp = transpose_128(ptr2, h2T[:, :, nt * P:(nt + 1) * P], "tph")
                nc.vector.tensor_tensor(
                    res_nat[:, nt, :],
                    tp.rearrange("p c q -> p (c q)"),
                    res_nat[:, nt, :],
                    op=OP.add,
                )
                nc.sync.dma_start(outp_v[:, nt, :], res_nat[:, nt, :])

    nc.compile()
    return nc


def _get_program():
    if "nc" not in _cache:
        _cache["nc"] = _build_program()
    return _cache["nc"]


def kernel(q0, kv0, normq_g, normq_b, normkv_g, normkv_b,
           Wq, Wk, Wv, Wr, mlp_g, mlp_b, W1, b1, W2, b2):
    q0 = np.asarray(q0, np.float32)
    kv0 = np.asarray(kv0, np.float32)
    f32 = lambda x: np.asarray(x, np.float32)
    normq_g, normq_b = f32(normq_g), f32(normq_b)
    normkv_g, normkv_b = f32(normkv_g), f32(normkv_b)
    mlp_g, mlp_b = f32(mlp_g), f32(mlp_b)
    Wq, Wk, Wv, Wr, W1, W2 = map(f32, (Wq, Wk, Wv, Wr, W1, W2))
    b1, b2 = f32(b1), f32(b2)

    # fold LN affine params into the following matmuls (exact: x_hat*g+b -> W)
    bf = lambda x: np.asarray(x, ml_dtypes.bfloat16)
    wq_e = normq_g[:, None] * Wq
    wk_e = normkv_g[:, None] * Wk
    wv_e = normkv_g[:, None] * Wv
    w1_e = mlp_g[:, None] * W1
    weights = {
        "wq": bf(wq_e), "wk": bf(wk_e), "wv": bf(wv_e),
        "wr": bf(Wr), "w1": bf(w1_e), "w2": bf(W2),
    }
    biases = {
        "bq": normq_b @ wq_e,
        "bk": normkv_b @ wk_e,
        "bv": normkv_b @ wv_e,
        "b1": (mlp_b @ w1_e) + b1,
        "b2": b2,
    }

    nc = _get_program()
    in_maps = []
    for core in range(NCORES):
        b, r = core // RSPLIT, core % RSPLIT
        m = {
            "q0r": np.ascontiguousarray(q0[b, r * NR:(r + 1) * NR]),
            "kv0b": np.ascontiguousarray(kv0[b]),
        }
        m.update(weights)
        m.update({k: np.ascontiguousarray(v) for k, v in biases.items()})
        in_maps.append(m)

    res = run_bass_kernel_spmd(nc, in_maps, list(range(NCORES)))
    out = np.empty((B, N, D), np.float32)
    for core in range(NCORES):
        b, r = core // RSPLIT, core % RSPLIT
        out[b, r * NR:(r + 1) * NR] = res.results[core]["out"]
    return out
